# revision 1
# baseline (speedup 1.0000x reference)
"""BiaffineAttn Trainium2 kernel.

Math (per batch b):
    t    = x2 @ U                      [S, D]
    attn = t @ x1^T + (x1 @ bias)[None, :]
    p    = softmax(attn, axis=-1)
    out  = relu((p @ x1) @ fc_w^T + fc_b)    [S, F]

Sharding: data-parallel over batch B=8, one batch per NeuronCore.

Per-core pipeline (all matmuls in fp32r = fp22-truncated fp32, which streams
at 1 cycle/row on the PE vs 4 for true fp32; N=512 moving chunks):
  The whole attention block is computed in TRANSPOSED orientation so that the
  softmax key dimension (t') lands on SBUF partitions:
    tT      = (x2 @ U)^T          stationary U[d,e] chunks, moving x2T[d,s]
    scoresT = attn^T [t', s]      stationary x1T[e,t'] 128x128 tiles, moving tT
    pT      = exp(scoresT - rowmax_bcast + kb)   (exact per-row max; kb is the
                                   per-key additive bias folded into exp's
                                   per-partition bias operand)
    oT      = (p @ x1)^T          stationary x1[t',e] chunks, moving pT
    outT    = relu((oT^T @ fcwT)^T * recip + fcb) stationary fcwT[e,f], moving oT
  rowmax: running elementwise max over the 16 t'-tiles of scoresT, then a
  128-partition reduce via 4 PE transposes, reassembled into a broadcast tile
  with K=1 ones-matmuls.  rowsum: ones-column matmuls accumulating over t'.

Host side: transposes x1/x2/fc_w per-core (layout prep for DMA-efficient
loads; fp32 DMA transpose does not exist on TRN2) and transposes the [F,S]
per-core output back to [S,F] when gathering.
"""

import os
import sys
from contextlib import ExitStack

import numpy as np

for _p in ("/opt/trn_rl_repo", os.path.expanduser("~/.axon_site/_ro/trn_rl_repo")):
    if os.path.isdir(_p) and _p not in sys.path:
        sys.path.insert(0, _p)

import concourse.bass as bass
import concourse.mybir as mybir
import concourse.tile as tile
from concourse import bacc

B = 8
S = 2048          # sequence length (both s and t')
D = 1024          # d_model
F = 512           # fc output dim
P = 128
SB = 512          # s superblock (moving free dim of every matmul)
NSB = S // SB     # 4
DC = D // P       # 8 contraction chunks of d / e
TC = S // P       # 16 t' tiles
FP32 = mybir.dt.float32
FP32R = mybir.dt.float32r
BF16 = mybir.dt.bfloat16
AF = mybir.ActivationFunctionType
ALU = mybir.AluOpType
AX = mybir.AxisListType


def build_nc():
    nc = bacc.Bacc(
        "TRN2",
        target_bir_lowering=False,
        debug=False,
        enable_asserts=False,
    )

    x1_d = nc.dram_tensor("x1", [S, D], BF16, kind="ExternalInput")
    x1t_d = nc.dram_tensor("x1t", [D, S], FP32R, kind="ExternalInput")
    x2t_d = nc.dram_tensor("x2t", [D, S], FP32R, kind="ExternalInput")
    u_d = nc.dram_tensor("u", [D, D], FP32R, kind="ExternalInput")
    fcwt_d = nc.dram_tensor("fcwt", [D, F], FP32R, kind="ExternalInput")
    bias_d = nc.dram_tensor("bias", [D, 1], FP32R, kind="ExternalInput")
    fcb_d = nc.dram_tensor("fcb", [F, 1], FP32, kind="ExternalInput")
    outt_d = nc.dram_tensor("outt", [F, S], FP32, kind="ExternalOutput")

    with tile.TileContext(nc) as tc, ExitStack() as ctx:
        # ---------- pools ----------
        p_u = ctx.enter_context(tc.tile_pool(name="ures", bufs=DC))
        p_x1 = ctx.enter_context(tc.tile_pool(name="x1res", bufs=TC))
        p_kb = ctx.enter_context(tc.tile_pool(name="kbcols", bufs=TC))
        p_bc = ctx.enter_context(tc.tile_pool(name="biascols", bufs=DC))
        p_fcb = ctx.enter_context(tc.tile_pool(name="fcbcols", bufs=F // P))
        p_ones = ctx.enter_context(tc.tile_pool(name="ones", bufs=1))
        p_psum = ctx.enter_context(tc.tile_pool(name="psum", bufs=8, space="PSUM"))
        p_x1tc = ctx.enter_context(tc.tile_pool(name="x1tcs", bufs=9))
        p_x2t = ctx.enter_context(tc.tile_pool(name="x2ts", bufs=8))
        p_tt = ctx.enter_context(tc.tile_pool(name="tts", bufs=8))
        p_sc = ctx.enter_context(tc.tile_pool(name="scores", bufs=TC))
        p_pb = ctx.enter_context(tc.tile_pool(name="pbf", bufs=TC))
        p_ot = ctx.enter_context(tc.tile_pool(name="ots", bufs=8))
        p_aux = ctx.enter_context(tc.tile_pool(name="aux", bufs=1))
        p_row = ctx.enter_context(tc.tile_pool(name="rows", bufs=1))
        p_out = ctx.enter_context(tc.tile_pool(name="outs", bufs=1))
        p_fcw = ctx.enter_context(tc.tile_pool(name="fcws", bufs=3))

        # ---------- small/fast inputs first: U, bias cols, identity ----------
        u_tiles = []
        for i in range(DC):
            u_t = p_u.tile([P, D], FP32R, name=f"ur{i}", tag="ur")
            nc.sync.dma_start(u_t[:], u_d[i * P : (i + 1) * P, :])
            u_tiles.append(u_t)
        bias_cols = []
        for i in range(DC):
            b_t = p_bc.tile([P, 1], FP32R, name=f"bc{i}", tag="bc")
            nc.sync.dma_start(b_t[:], bias_d[i * P : (i + 1) * P, :])
            bias_cols.append(b_t)
        fcb_cols = []
        for i in range(F // P):
            c_t = p_fcb.tile([P, 1], FP32, name=f"fcb{i}", tag="fcb")
            nc.sync.dma_start(c_t[:], fcb_d[i * P : (i + 1) * P, :])
            fcb_cols.append(c_t)
        identity = p_ones.tile([P, P], FP32, name="ident", tag="ident")
        nc.gpsimd.memset(identity[:], 0.0)
        nc.gpsimd.affine_select(
            out=identity[:], in_=identity[:], compare_op=ALU.not_equal,
            fill=1.0, base=0, pattern=[[-1, P]], channel_multiplier=1,
        )
        ones_row = p_ones.tile([1, P], FP32R, name="ones_row", tag="ones_row")
        nc.scalar.activation(ones_row[:], identity[0:1, :], AF.Identity, bias=1.0, scale=0.0)

        def load_x1t_chunks(tg, who):
            chunks = []
            for ec in range(DC):
                c = p_x1tc.tile([P, SB], FP32R, name=f"x1tc_{who}_{tg}_{ec}", tag="x1tc")
                nc.sync.dma_start(
                    c[:], x1t_d[ec * P : (ec + 1) * P, tg * SB : (tg + 1) * SB]
                )
                chunks.append(c)
            return chunks

        # ---------- kb prepass: kb = x1 @ bias as per-t' columns ----------
        # kb_row[tg] = bias_colchunks.T @ x1t chunks; then K=1 transposes to cols.
        kb_rows = []
        for tg in range(S // SB):
            chunks = load_x1t_chunks(tg, "kb")
            ps_kb = p_psum.tile([1, SB], FP32, name=f"pskb{tg}", tag="ps")
            for ec in range(DC):
                nc.tensor.matmul(
                    ps_kb[:], bias_cols[ec][:], chunks[ec][:],
                    start=(ec == 0), stop=(ec == DC - 1),
                )
            kb_r = p_row.tile([1, SB], FP32R, name=f"kbrow{tg}", tag=f"kbrow{tg}")
            nc.vector.tensor_copy(kb_r[:], ps_kb[:])
            kb_rows.append(kb_r)
        kb_cols = []
        for ti in range(TC):
            tg, sub = ti // 4, ti % 4
            ps_c = p_psum.tile([P, 4], FP32, name=f"pskc{ti}", tag="ps")
            nc.tensor.matmul(
                ps_c[:], kb_rows[tg][0:1, sub * P : (sub + 1) * P],
                ones_row[0:1, 0:4], start=True, stop=True,
            )
            kb_c = p_kb.tile([P, 1], FP32, name=f"kb{ti}", tag="kb")
            nc.vector.tensor_copy(kb_c[:], ps_c[:, 0:1])
            kb_cols.append(kb_c)

        # ---------- MM1 emitter ----------
        def emit_mm1(sb):
            s0 = sb * SB
            x2t_tiles = []
            for dc in range(DC):
                x2_t = p_x2t.tile([P, SB], FP32R, name=f"x2t_{sb}_{dc}", tag="x2t")
                nc.sync.dma_start(x2_t[:], x2t_d[dc * P : (dc + 1) * P, s0 : s0 + SB])
                x2t_tiles.append(x2_t)
            tt_tiles = []
            for eh in range(2):
                ps_t = [
                    p_psum.tile([P, SB], FP32, name=f"pst{sb}_{eh}_{i}", tag="ps")
                    for i in range(4)
                ]
                for dc in range(DC):
                    for i in range(4):
                        et = eh * 4 + i
                        nc.tensor.matmul(
                            ps_t[i][:],
                            u_tiles[dc][:, et * P : (et + 1) * P],
                            x2t_tiles[dc][:],
                            start=(dc == 0), stop=(dc == DC - 1),
                        )
                for i in range(4):
                    t_t = p_tt.tile([P, SB], FP32R, name=f"tt{sb}_{eh}_{i}", tag="tt")
                    nc.vector.tensor_copy(t_t[:], ps_t[i][:])
                    tt_tiles.append(t_t)
            return tt_tiles

        next_tt = emit_mm1(0)

        # ---------- resident x1 (bf16) for MM4 stationaries ----------
        x1_tiles = []
        for i in range(TC):
            x1_t = p_x1.tile([P, D], BF16, name=f"x1r{i}", tag="x1r")
            nc.sync.dma_start(x1_t[:], x1_d[i * P : (i + 1) * P, :])
            x1_tiles.append(x1_t)

        for sb in range(NSB):
            s0 = sb * SB
            tt_tiles = next_tt

            # ---- MM2: scoresT tiles + running elementwise max ----
            sc_tiles = []
            maxacc = p_aux.tile([P, SB], FP32, name=f"maxacc{sb}", tag="maxacc")
            chunks = None
            for ti in range(TC):
                tg, sub = ti // 4, ti % 4
                if sub == 0:
                    chunks = load_x1t_chunks(tg, f"s{sb}")
                ps_s = p_psum.tile([P, SB], FP32, name=f"pss{sb}_{ti}", tag="ps")
                for ec in range(DC):
                    nc.tensor.matmul(
                        ps_s[:],
                        chunks[ec][:, sub * P : (sub + 1) * P],
                        tt_tiles[ec][:],
                        start=(ec == 0), stop=(ec == DC - 1),
                    )
                s_t = p_sc.tile([P, SB], FP32, name=f"sc{sb}_{ti}", tag="sc")
                nc.vector.tensor_copy(s_t[:], ps_s[:])
                if ti == 0:
                    nc.scalar.copy(maxacc[:], ps_s[:])
                else:
                    nc.vector.tensor_max(maxacc[:], maxacc[:], ps_s[:])
                sc_tiles.append(s_t)

            # pipeline: next superblock's MM1 runs during the softmax phase
            if sb + 1 < NSB:
                next_tt = emit_mm1(sb + 1)

            # ---- per-s max over partitions: transpose + free reduce ----
            mrow = p_row.tile([1, SB], FP32R, name=f"mrow{sb}", tag="mrow")
            for blk in range(SB // P):
                ps_tr = p_psum.tile([P, P], FP32, name=f"ptr{sb}_{blk}", tag="ps")
                nc.tensor.transpose(
                    ps_tr[:], maxacc[:, blk * P : (blk + 1) * P], identity[:]
                )
                mcol = p_row.tile([P, 1], FP32, name=f"mcol{sb}_{blk}", tag="mcol")
                nc.vector.reduce_max(mcol[:], ps_tr[:], axis=AX.X)
                ps_rr = p_psum.tile([1, P], FP32, name=f"prr{sb}_{blk}", tag="ps")
                nc.tensor.transpose(ps_rr[:], mcol[:], identity[:])
                nc.vector.tensor_copy(mrow[:, blk * P : (blk + 1) * P], ps_rr[:])
            ps_mb = p_psum.tile([P, SB], FP32, name=f"pmb{sb}", tag="ps")
            nc.tensor.matmul(ps_mb[:], ones_row[:], mrow[:], start=True, stop=True)
            maxb = p_aux.tile([P, SB], FP32, name=f"maxb{sb}", tag="maxb")
            nc.vector.tensor_copy(maxb[:], ps_mb[:])

            # ---- exp(scores - maxb + kb) -> bf16 p tiles; running sum ----
            pb_tiles = []
            sumacc = p_aux.tile([P, SB], FP32, name=f"sumacc{sb}", tag="sumacc")
            for ti in range(TC):
                nc.vector.tensor_sub(sc_tiles[ti][:], sc_tiles[ti][:], maxb[:])
                p_t = p_pb.tile([P, SB], BF16, name=f"pb{sb}_{ti}", tag="pb")
                nc.scalar.activation(
                    p_t[:], sc_tiles[ti][:], AF.Exp, bias=kb_cols[ti][:], scale=1.0
                )
                if ti == 0:
                    nc.scalar.copy(sumacc[:], p_t[:])
                else:
                    nc.vector.tensor_add(sumacc[:], sumacc[:], p_t[:])
                pb_tiles.append(p_t)

            # ---- per-s sum over partitions: transpose + free reduce ----
            srow = p_row.tile([1, SB], FP32, name=f"srow{sb}", tag="srow")
            for blk in range(SB // P):
                ps_tr2 = p_psum.tile([P, P], FP32, name=f"ptr2{sb}_{blk}", tag="ps")
                nc.tensor.transpose(
                    ps_tr2[:], sumacc[:, blk * P : (blk + 1) * P], identity[:]
                )
                scol = p_row.tile([P, 1], FP32, name=f"scol{sb}_{blk}", tag="scol")
                nc.vector.reduce_sum(scol[:], ps_tr2[:], axis=AX.X)
                ps_rr2 = p_psum.tile([1, P], FP32, name=f"prr2{sb}_{blk}", tag="ps")
                nc.tensor.transpose(ps_rr2[:], scol[:], identity[:])
                nc.vector.tensor_copy(srow[:, blk * P : (blk + 1) * P], ps_rr2[:])
            rrow = p_row.tile([1, SB], FP32R, name=f"rrow{sb}", tag="rrow")
            with nc.allow_low_precision(reason="recip feeds fp32r matmul; fp22 ok"):
                nc.vector.reciprocal(rrow[:], srow[:])

            # ---- MM4 (bf16): oT = (p~ @ x1)^T ----
            ot_tiles = []
            for et in range(DC):
                ps_o = p_psum.tile([P, SB], FP32, name=f"pso{sb}_{et}", tag="ps")
                for ti in range(TC):
                    nc.tensor.matmul(
                        ps_o[:],
                        x1_tiles[ti][:, et * P : (et + 1) * P],
                        pb_tiles[ti][:],
                        start=(ti == 0), stop=(ti == TC - 1),
                    )
                o_t = p_ot.tile([P, SB], FP32R, name=f"ot{sb}_{et}", tag="ot")
                nc.vector.tensor_copy(o_t[:], ps_o[:])
                ot_tiles.append(o_t)

            # recip broadcast (emitted after MM4 so the PE isn't head-of-line
            # blocked on the DVE reciprocal)
            ps_rb = p_psum.tile([P, SB], FP32, name=f"prb{sb}", tag="ps")
            nc.tensor.matmul(ps_rb[:], ones_row[:], rrow[:], start=True, stop=True)
            recipb = p_aux.tile([P, SB], FP32, name=f"recipb{sb}", tag="recipb")
            nc.vector.tensor_copy(recipb[:], ps_rb[:])

            # ---- MM5 + normalize + bias + relu + store ----
            for ft in range(F // P):
                ps_f = p_psum.tile([P, SB], FP32, name=f"psf{sb}_{ft}", tag="ps")
                for ec in range(DC):
                    fcw_t = p_fcw.tile([P, P], FP32R, name=f"fcw{sb}_{ft}_{ec}", tag="fcw")
                    nc.sync.dma_start(
                        fcw_t[:], fcwt_d[ec * P : (ec + 1) * P, ft * P : (ft + 1) * P]
                    )
                    nc.tensor.matmul(
                        ps_f[:], fcw_t[:], ot_tiles[ec][:],
                        start=(ec == 0), stop=(ec == DC - 1),
                    )
                tmp = p_out.tile([P, SB], FP32, name=f"tmp{sb}_{ft}", tag="tmp")
                nc.vector.tensor_mul(tmp[:], ps_f[:], recipb[:])
                o_out = p_out.tile([P, SB], FP32, name=f"oo{sb}_{ft}", tag="oo")
                nc.scalar.activation(
                    o_out[:], tmp[:], AF.Relu, bias=fcb_cols[ft][:], scale=1.0
                )
                nc.sync.dma_start(outt_d[ft * P : (ft + 1) * P, s0 : s0 + SB], o_out[:])

    nc.compile()
    return nc


_NC_CACHE = None


def _get_nc():
    global _NC_CACHE
    if _NC_CACHE is None:
        _NC_CACHE = build_nc()
    return _NC_CACHE


def make_in_maps(x1, x2, U, bias, fc_w, fc_b):
    import ml_dtypes

    x1 = np.ascontiguousarray(np.asarray(x1, dtype=np.float32))
    x2 = np.ascontiguousarray(np.asarray(x2, dtype=np.float32))
    U = np.ascontiguousarray(np.asarray(U, dtype=np.float32))
    bias = np.asarray(bias, dtype=np.float32).reshape(D, 1)
    fc_w = np.asarray(fc_w, dtype=np.float32)
    fc_b = np.asarray(fc_b, dtype=np.float32).reshape(F, 1)
    fcwt = np.ascontiguousarray(fc_w.T)
    in_maps = []
    for b in range(B):
        in_maps.append(
            {
                "x1": np.ascontiguousarray(x1[b].astype(ml_dtypes.bfloat16)),
                "x1t": np.ascontiguousarray(x1[b].T),
                "x2t": np.ascontiguousarray(x2[b].T),
                "u": U,
                "fcwt": fcwt,
                "bias": bias,
                "fcb": fc_b,
            }
        )
    return in_maps


def kernel(x1, x2, U, bias, fc_w, fc_b):
    from concourse.bass_utils import run_bass_kernel_spmd

    nc = _get_nc()
    in_maps = make_in_maps(x1, x2, U, bias, fc_w, fc_b)
    res = run_bass_kernel_spmd(nc, in_maps, core_ids=list(range(B)))
    out = np.stack([np.ascontiguousarray(r["outt"].T) for r in res.results])
    return out.astype(np.float32)



# revision 4
# speedup vs baseline: 1.5364x; 1.5364x over previous
"""BiaffineAttn Trainium2 kernel.

Math (per batch b):
    t    = x2 @ U                      [S, D]
    attn = t @ x1^T + (x1 @ bias)[None, :]
    p    = softmax(attn, axis=-1)
    out  = relu((p @ x1) @ fc_w^T + fc_b)    [S, F]

Sharding: data-parallel over batch B=8, one batch per NeuronCore.

Key algebraic restructure vs the naive form: (p @ x1) @ fc_w^T = p @ (x1 @
fc_w^T), so a one-time prepass x1fc = x1 @ fc_w^T [S, F] (bf16) replaces the
per-superblock S*S*D + S*D*F output matmuls with S*S*F — 40% fewer MACs on
the output side.

Per-core pipeline (attention in TRANSPOSED orientation so the softmax key
dimension t' lands on SBUF partitions; N=512 moving chunks; fp32r = fp22
matmuls except the bf16 p-path):
    tT      = (x2 @ U)^T            stationary U[d,e] chunks, moving x2T[d,s]
    scoresT = attn^T [t', s]        stationary x1T[e,t'] 128x128 tiles
    pT      = exp(scoresT - rowmax_bcast + kb)  (kb = x1 @ bias folded into
                                    exp's per-partition bias operand)
    rowsum  = ones_col^T @ pT       PE K-accumulated 1-row matmul (no DVE
                                    reduce pipeline)
    outT    = relu(recip * (x1fc^T @ pT) + fcb)   stationary x1fc tiles
  rowmax: gpsimd elementwise-max chase over the 16 t'-tiles, then a
  128-partition reduce via PE transposes + ones-matmul broadcast.

Engine placement: PSUM->SBUF score copies on ScalarE, max-chase and the
(scores - max) subtract on GpSimdE, so VectorE only handles small reduces,
copies and the final normalize — the PE never waits on a vector pipeline.

The exp phase is chased by the PE (rowsum + output matmuls per tile) and
backfilled with the next superblock's tT matmuls, so the tensor engine never
idles long enough for the HAM clock gate to re-throttle it.

Host side: transposes x1/x2/fc_w per-core (fp32 DMA transpose does not exist
on TRN2) and transposes the [F,S] per-core output back to [S,F].
"""

import os
import sys
from contextlib import ExitStack

import numpy as np

for _p in ("/opt/trn_rl_repo", os.path.expanduser("~/.axon_site/_ro/trn_rl_repo")):
    if os.path.isdir(_p) and _p not in sys.path:
        sys.path.insert(0, _p)

import concourse.bass as bass
import concourse.mybir as mybir
import concourse.tile as tile
from concourse import bacc

B = 8
S = 2048          # sequence length (both s and t')
D = 1024          # d_model
F = 512           # fc output dim
P = 128
SB = 512          # s superblock (moving free dim of every matmul)
NSB = S // SB     # 4
DC = D // P       # 8 contraction chunks of d / e
TC = S // P       # 16 t' tiles
FPG = F // P      # 4 output row-tiles
FP32 = mybir.dt.float32
FP32R = mybir.dt.float32r
BF16 = mybir.dt.bfloat16
AF = mybir.ActivationFunctionType
ALU = mybir.AluOpType
AX = mybir.AxisListType


def build_nc():
    nc = bacc.Bacc(
        "TRN2",
        target_bir_lowering=False,
        debug=False,
        enable_asserts=False,
    )

    x1t_d = nc.dram_tensor("x1t", [D, S], FP32R, kind="ExternalInput")
    x2t_d = nc.dram_tensor("x2t", [D, S], FP32R, kind="ExternalInput")
    u_d = nc.dram_tensor("u", [D, D], FP32R, kind="ExternalInput")
    fcwt_d = nc.dram_tensor("fcwt", [D, F], FP32R, kind="ExternalInput")
    bias_d = nc.dram_tensor("bias", [D, 1], FP32R, kind="ExternalInput")
    fcb_d = nc.dram_tensor("fcb", [F, 1], FP32, kind="ExternalInput")
    outt_d = nc.dram_tensor("outt", [F, S], FP32, kind="ExternalOutput")

    with tile.TileContext(nc) as tc, ExitStack() as ctx:
        # ---------- pools ----------
        p_u = ctx.enter_context(tc.tile_pool(name="ures", bufs=DC))
        p_fcw = ctx.enter_context(tc.tile_pool(name="fcwres", bufs=DC))
        p_x1fc = ctx.enter_context(tc.tile_pool(name="x1fcs", bufs=TC))
        p_kb = ctx.enter_context(tc.tile_pool(name="kbcols", bufs=TC))
        p_bc = ctx.enter_context(tc.tile_pool(name="biascols", bufs=DC))
        p_fcb = ctx.enter_context(tc.tile_pool(name="fcbcols", bufs=FPG))
        p_ones = ctx.enter_context(tc.tile_pool(name="ones", bufs=1))
        p_x1tc = ctx.enter_context(tc.tile_pool(name="x1tcs", bufs=16))
        p_x2t = ctx.enter_context(tc.tile_pool(name="x2ts", bufs=8))
        p_tt = ctx.enter_context(tc.tile_pool(name="tts", bufs=8))
        p_sc = ctx.enter_context(tc.tile_pool(name="scores", bufs=TC))
        p_pb = ctx.enter_context(tc.tile_pool(name="pbf", bufs=TC))
        p_aux = ctx.enter_context(tc.tile_pool(name="aux", bufs=1))
        p_row = ctx.enter_context(tc.tile_pool(name="rows", bufs=1))
        p_out = ctx.enter_context(tc.tile_pool(name="outs", bufs=2))
        # PSUM: 4 banks for the output accumulators, 1 for the rowsum, 3
        # general-purpose (MM1/MM2/transposes/broadcasts) = 8 banks exactly.
        p_pso = ctx.enter_context(tc.tile_pool(name="pso", bufs=FPG, space="PSUM"))
        p_pssum = ctx.enter_context(tc.tile_pool(name="pssum", bufs=1, space="PSUM"))
        p_psg = ctx.enter_context(tc.tile_pool(name="psg", bufs=3, space="PSUM"))

        # ---------- resident small inputs ----------
        u_tiles = []
        for i in range(DC):
            u_t = p_u.tile([P, D], FP32R, name=f"ur{i}", tag="ur")
            nc.sync.dma_start(u_t[:], u_d[i * P : (i + 1) * P, :])
            u_tiles.append(u_t)
        fcw_tiles = []
        for i in range(DC):
            f_t = p_fcw.tile([P, F], FP32R, name=f"fcw{i}", tag="fcw")
            nc.sync.dma_start(f_t[:], fcwt_d[i * P : (i + 1) * P, :])
            fcw_tiles.append(f_t)
        bias_cols = []
        for i in range(DC):
            b_t = p_bc.tile([P, 1], FP32R, name=f"bc{i}", tag="bc")
            nc.sync.dma_start(b_t[:], bias_d[i * P : (i + 1) * P, :])
            bias_cols.append(b_t)
        fcb_cols = []
        for i in range(FPG):
            c_t = p_fcb.tile([P, 1], FP32, name=f"fcb{i}", tag="fcb")
            nc.sync.dma_start(c_t[:], fcb_d[i * P : (i + 1) * P, :])
            fcb_cols.append(c_t)
        identity = p_ones.tile([P, P], FP32, name="ident", tag="ident")
        nc.gpsimd.memset(identity[:], 0.0)
        nc.gpsimd.affine_select(
            out=identity[:], in_=identity[:], compare_op=ALU.not_equal,
            fill=1.0, base=0, pattern=[[-1, P]], channel_multiplier=1,
        )
        ones_row = p_ones.tile([1, P], FP32R, name="ones_row", tag="ones_row")
        nc.scalar.activation(ones_row[:], identity[0:1, :], AF.Identity, bias=1.0, scale=0.0)
        ones_col = p_ones.tile([P, 1], BF16, name="ones_col", tag="ones_col")
        nc.scalar.activation(ones_col[:], identity[:, 0:1], AF.Identity, bias=1.0, scale=0.0)

        # ---------- MM1 emitter: tT chunks for superblock sb ----------
        def emit_mm1(sb):
            s0 = sb * SB
            x2t_tiles = []
            for dc in range(DC):
                x2_t = p_x2t.tile([P, SB], FP32R, name=f"x2t_{sb}_{dc}", tag="x2t")
                nc.sync.dma_start(x2_t[:], x2t_d[dc * P : (dc + 1) * P, s0 : s0 + SB])
                x2t_tiles.append(x2_t)
            tt_tiles = []
            for g in range(4):
                ps_t = [
                    p_psg.tile([P, SB], FP32, name=f"pst{sb}_{g}_{i}", tag="psg")
                    for i in range(2)
                ]
                for dc in range(DC):
                    for i in range(2):
                        et = g * 2 + i
                        nc.tensor.matmul(
                            ps_t[i][:],
                            u_tiles[dc][:, et * P : (et + 1) * P],
                            x2t_tiles[dc][:],
                            start=(dc == 0), stop=(dc == DC - 1),
                        )
                for i in range(2):
                    t_t = p_tt.tile([P, SB], FP32R, name=f"tt{sb}_{g}_{i}", tag="tt")
                    nc.vector.tensor_copy(t_t[:], ps_t[i][:])
                    tt_tiles.append(t_t)
            return tt_tiles

        next_tt = emit_mm1(0)

        kb_cols = [None] * TC
        x1fc_tiles = [None] * TC

        for sb in range(NSB):
            s0 = sb * SB
            tt_tiles = next_tt

            # ---- MM2: scoresT tiles + running elementwise max ----
            # sb 0 also folds in the x1fc and kb prepasses, which reuse the
            # same x1t chunks while they are resident.
            sc_tiles = []
            maxacc = p_aux.tile([P, SB], FP32, name=f"maxacc{sb}", tag="maxacc")
            for tg in range(S // SB):
                chunks = []
                for ec in range(DC):
                    c = p_x1tc.tile([P, SB], FP32R, name=f"x1tc_{sb}_{tg}_{ec}", tag="x1tc")
                    nc.sync.dma_start(
                        c[:], x1t_d[ec * P : (ec + 1) * P, tg * SB : (tg + 1) * SB]
                    )
                    chunks.append(c)
                for sub in range(4):
                    ti = tg * 4 + sub
                    ps_s = p_psg.tile([P, SB], FP32, name=f"pss{sb}_{ti}", tag="psg")
                    for ec in range(DC):
                        nc.tensor.matmul(
                            ps_s[:],
                            chunks[ec][:, sub * P : (sub + 1) * P],
                            tt_tiles[ec][:],
                            start=(ec == 0), stop=(ec == DC - 1),
                        )
                    s_t = p_sc.tile([P, SB], FP32, name=f"sc{sb}_{ti}", tag="sc")
                    nc.scalar.copy(s_t[:], ps_s[:])
                    if ti == 0:
                        nc.gpsimd.tensor_copy(maxacc[:], s_t[:])
                    else:
                        nc.vector.tensor_max(maxacc[:], maxacc[:], s_t[:])
                    sc_tiles.append(s_t)
                if sb == 0:
                    # x1fc[ti] = x1 @ fc_w^T tile  [t'128, F] (bf16)
                    for sub in range(4):
                        ti = tg * 4 + sub
                        ps_x = p_psg.tile([P, F], FP32, name=f"psx{ti}", tag="psg")
                        for ec in range(DC):
                            nc.tensor.matmul(
                                ps_x[:],
                                chunks[ec][:, sub * P : (sub + 1) * P],
                                fcw_tiles[ec][:],
                                start=(ec == 0), stop=(ec == DC - 1),
                            )
                        xf = p_x1fc.tile([P, F], BF16, name=f"x1fc{ti}", tag="x1fc")
                        nc.vector.tensor_copy(xf[:], ps_x[:])
                        x1fc_tiles[ti] = xf
                    # kb rows for this tg: bias^T @ x1t -> [1, SB], then
                    # spread to per-t' columns via K=1 ones-matmuls.
                    ps_kb = p_pssum.tile([1, SB], FP32, name=f"pskb{tg}", tag="pssum")
                    for ec in range(DC):
                        nc.tensor.matmul(
                            ps_kb[:], bias_cols[ec][:], chunks[ec][:],
                            start=(ec == 0), stop=(ec == DC - 1),
                        )
                    kb_r = p_row.tile([1, SB], FP32R, name=f"kbrow{tg}", tag=f"kbrow{tg}")
                    nc.vector.tensor_copy(kb_r[:], ps_kb[:])
                    for sub in range(4):
                        ti = tg * 4 + sub
                        ps_c = p_psg.tile([P, 4], FP32, name=f"pskc{ti}", tag="psg")
                        nc.tensor.matmul(
                            ps_c[:], kb_r[0:1, sub * P : (sub + 1) * P],
                            ones_row[0:1, 0:4], start=True, stop=True,
                        )
                        kb_c = p_kb.tile([P, 1], FP32, name=f"kb{ti}", tag="kb")
                        nc.vector.tensor_copy(kb_c[:], ps_c[:, 0:1])
                        kb_cols[ti] = kb_c

            # ---- per-s max over partitions: transpose + free reduce ----
            mrow = p_row.tile([1, SB], FP32R, name=f"mrow{sb}", tag="mrow")
            for blk in range(SB // P):
                ps_tr = p_psg.tile([P, P], FP32, name=f"ptr{sb}_{blk}", tag="psg")
                nc.tensor.transpose(
                    ps_tr[:], maxacc[:, blk * P : (blk + 1) * P], identity[:]
                )
                mcol = p_row.tile([P, 1], FP32, name=f"mcol{sb}_{blk}", tag="mcol")
                nc.vector.reduce_max(mcol[:], ps_tr[:], axis=AX.X)
                ps_rr = p_psg.tile([1, P], FP32, name=f"prr{sb}_{blk}", tag="psg")
                nc.tensor.transpose(ps_rr[:], mcol[:], identity[:])
                nc.vector.tensor_copy(mrow[:, blk * P : (blk + 1) * P], ps_rr[:])
            ps_mb = p_psg.tile([P, SB], FP32, name=f"pmb{sb}", tag="psg")
            nc.tensor.matmul(ps_mb[:], ones_row[:], mrow[:], start=True, stop=True)
            maxb = p_aux.tile([P, SB], FP32, name=f"maxb{sb}", tag="maxb")
            nc.vector.tensor_copy(maxb[:], ps_mb[:])

            # ---- exp phase, chased by the PE: rowsum + output matmuls ----
            ps_sum = p_pssum.tile([1, SB], FP32, name=f"pssm{sb}", tag="pssum")
            ps_o = [
                p_pso.tile([P, SB], FP32, name=f"pso{sb}_{ft}", tag="pso")
                for ft in range(FPG)
            ]
            for ti in range(TC):
                nc.gpsimd.tensor_sub(sc_tiles[ti][:], sc_tiles[ti][:], maxb[:])
                p_t = p_pb.tile([P, SB], BF16, name=f"pb{sb}_{ti}", tag="pb")
                nc.scalar.activation(
                    p_t[:], sc_tiles[ti][:], AF.Exp, bias=kb_cols[ti][:], scale=1.0
                )
                nc.tensor.matmul(
                    ps_sum[:], ones_col[:], p_t[:],
                    start=(ti == 0), stop=(ti == TC - 1),
                )
                for ft in range(FPG):
                    nc.tensor.matmul(
                        ps_o[ft][:],
                        x1fc_tiles[ti][:, ft * P : (ft + 1) * P],
                        p_t[:],
                        start=(ti == 0), stop=(ti == TC - 1),
                    )

            # next superblock's MM1 backfills the PE behind the exp chase
            if sb + 1 < NSB:
                next_tt = emit_mm1(sb + 1)

            # ---- recip + broadcast ----
            srow = p_row.tile([1, SB], FP32, name=f"srow{sb}", tag="srow")
            nc.vector.tensor_copy(srow[:], ps_sum[:])
            rrow = p_row.tile([1, SB], FP32R, name=f"rrow{sb}", tag="rrow")
            with nc.allow_low_precision(reason="recip feeds fp32r matmul; fp22 ok"):
                nc.vector.reciprocal(rrow[:], srow[:])
            ps_rb = p_psg.tile([P, SB], FP32, name=f"prb{sb}", tag="psg")
            nc.tensor.matmul(ps_rb[:], ones_row[:], rrow[:], start=True, stop=True)
            recipb = p_aux.tile([P, SB], FP32, name=f"recipb{sb}", tag="recipb")
            nc.vector.tensor_copy(recipb[:], ps_rb[:])

            # ---- normalize + bias + relu + store ----
            for ft in range(FPG):
                tmp = p_out.tile([P, SB], FP32, name=f"tmp{sb}_{ft}", tag="tmp")
                nc.vector.tensor_mul(tmp[:], ps_o[ft][:], recipb[:])
                o_out = p_out.tile([P, SB], FP32, name=f"oo{sb}_{ft}", tag="oo")
                nc.scalar.activation(
                    o_out[:], tmp[:], AF.Relu, bias=fcb_cols[ft][:], scale=1.0
                )
                nc.sync.dma_start(outt_d[ft * P : (ft + 1) * P, s0 : s0 + SB], o_out[:])

    nc.compile()
    return nc


_NC_CACHE = None


def _get_nc():
    global _NC_CACHE
    if _NC_CACHE is None:
        _NC_CACHE = build_nc()
    return _NC_CACHE


def make_in_maps(x1, x2, U, bias, fc_w, fc_b):
    x1 = np.ascontiguousarray(np.asarray(x1, dtype=np.float32))
    x2 = np.ascontiguousarray(np.asarray(x2, dtype=np.float32))
    U = np.ascontiguousarray(np.asarray(U, dtype=np.float32))
    bias = np.asarray(bias, dtype=np.float32).reshape(D, 1)
    fc_w = np.asarray(fc_w, dtype=np.float32)
    fc_b = np.asarray(fc_b, dtype=np.float32).reshape(F, 1)
    fcwt = np.ascontiguousarray(fc_w.T)
    in_maps = []
    for b in range(B):
        in_maps.append(
            {
                "x1t": np.ascontiguousarray(x1[b].T),
                "x2t": np.ascontiguousarray(x2[b].T),
                "u": U,
                "fcwt": fcwt,
                "bias": bias,
                "fcb": fc_b,
            }
        )
    return in_maps


def kernel(x1, x2, U, bias, fc_w, fc_b):
    from concourse.bass_utils import run_bass_kernel_spmd

    nc = _get_nc()
    in_maps = make_in_maps(x1, x2, U, bias, fc_w, fc_b)
    res = run_bass_kernel_spmd(nc, in_maps, core_ids=list(range(B)))
    out = np.stack([np.ascontiguousarray(r["outt"].T) for r in res.results])
    return out.astype(np.float32)


# revision 6
# speedup vs baseline: 1.6120x; 1.0493x over previous
"""BiaffineAttn Trainium2 kernel.

Math (per batch b):
    t    = x2 @ U                      [S, D]
    attn = t @ x1^T + (x1 @ bias)[None, :]
    p    = softmax(attn, axis=-1)
    out  = relu((p @ x1) @ fc_w^T + fc_b)    [S, F]

Sharding: data-parallel over batch B=8, one batch per NeuronCore.

Key algebraic restructure vs the naive form: (p @ x1) @ fc_w^T = p @ (x1 @
fc_w^T), so a one-time prepass x1fc = x1 @ fc_w^T [S, F] (bf16) replaces the
per-superblock S*S*D + S*D*F output matmuls with S*S*F — 40% fewer MACs on
the output side.

Per-core pipeline (attention in TRANSPOSED orientation so the softmax key
dimension t' lands on SBUF partitions; N=512 moving chunks):
    tT      = (x2 @ U)^T            fp32r; stationary U chunks, moving x2T
    scoresT = attn^T [t', s]        fp32r x1T 128x128 stationaries, fp32r tT
    pT      = exp(scoresT - rowmax_bcast + kb)  (kb = x1 @ bias folded into
                                    exp's per-partition bias operand)
    rowsum  = ones_col^T @ pT       PE K-accumulated 1-row matmul (no DVE
                                    reduce pipeline)
    outT    = relu(recip * (x1fc^T @ pT) + fcb)   stationary x1fc tiles
  rowmax: elementwise-max chase over the 16 t'-tiles on VectorE, then a
  128-partition reduce via PE transposes + ones-matmul broadcast.

The score path stays fp32r (fp22) end-to-end: with scores ~N(0, 32^2) the
softmax is highly peaked and any bf16 rounding upstream (~0.2 absolute on
scores) turns into ~20% errors on near-tie attention weights, blowing the
max-abs error metric.  bf16 is confined to the post-exp path (p, x1fc),
where errors average out over the 2048-key contraction.

x1T lives fp32r-resident in SBUF (8MB) — loaded once, reused by the kb/x1fc
prepasses and all four superblocks' score matmuls; no steady-state reloads.

Schedule: the kb/x1fc prepasses are the first PE work (they only need ~3MB
of DMA), warming the HAM clock gate while U/x2T stream in. Each
superblock's exp phase is chased by the PE (rowsum + output matmuls per
tile); the next superblock's tT matmuls are emitted in two halves around the
chase to cover the max-reduce and recip handoff bubbles, so the tensor
engine never idles long enough to re-throttle.

Engine placement: PSUM->SBUF score copies on ScalarE, the (scores - max)
subtract on GpSimdE, max-chase/reduces/normalize on VectorE.

SBUF packing: the 8 fc_w chunks are allocated from the tT pool's ring (they
die in the prepass, before the first tT tile exists), x2T chunk DMAs for
superblock 0 are hoisted to kernel start, and the p ring holds 6 tiles.

Host side: transposes x1/x2/fc_w per-core (fp32 DMA transpose does not exist
on TRN2) and transposes the [F,S] per-core output back to [S,F].
"""

import os
import sys
from contextlib import ExitStack

import numpy as np

for _p in ("/opt/trn_rl_repo", os.path.expanduser("~/.axon_site/_ro/trn_rl_repo")):
    if os.path.isdir(_p) and _p not in sys.path:
        sys.path.insert(0, _p)

import concourse.bass as bass
import concourse.mybir as mybir
import concourse.tile as tile
from concourse import bacc

B = 8
S = 2048          # sequence length (both s and t')
D = 1024          # d_model
F = 512           # fc output dim
P = 128
SB = 512          # s superblock (moving free dim of every matmul)
NSB = S // SB     # 4
DC = D // P       # 8 contraction chunks of d / e
TC = S // P       # 16 t' tiles
FPG = F // P      # 4 output row-tiles
FP32 = mybir.dt.float32
FP32R = mybir.dt.float32r
BF16 = mybir.dt.bfloat16
AF = mybir.ActivationFunctionType
ALU = mybir.AluOpType
AX = mybir.AxisListType


def build_nc():
    nc = bacc.Bacc(
        "TRN2",
        target_bir_lowering=False,
        debug=False,
        enable_asserts=False,
    )

    x1t_d = nc.dram_tensor("x1t", [D, S], FP32R, kind="ExternalInput")
    x2t_d = nc.dram_tensor("x2t", [D, S], FP32R, kind="ExternalInput")
    u_d = nc.dram_tensor("u", [D, D], FP32R, kind="ExternalInput")
    fcwt_d = nc.dram_tensor("fcwt", [D, F], FP32R, kind="ExternalInput")
    bias_d = nc.dram_tensor("bias", [D, 1], FP32R, kind="ExternalInput")
    fcb_d = nc.dram_tensor("fcb", [F, 1], FP32, kind="ExternalInput")
    outt_d = nc.dram_tensor("outt", [F, S], FP32, kind="ExternalOutput")

    with tile.TileContext(nc) as tc, ExitStack() as ctx:
        # ---------- pools ----------
        p_u = ctx.enter_context(tc.tile_pool(name="ures", bufs=DC))
        p_x1fc = ctx.enter_context(tc.tile_pool(name="x1fcs", bufs=TC))
        p_kb = ctx.enter_context(tc.tile_pool(name="kbcols", bufs=TC))
        p_bc = ctx.enter_context(tc.tile_pool(name="biascols", bufs=DC))
        p_fcb = ctx.enter_context(tc.tile_pool(name="fcbcols", bufs=FPG))
        p_ones = ctx.enter_context(tc.tile_pool(name="ones", bufs=1))
        p_x1tc = ctx.enter_context(tc.tile_pool(name="x1tcs", bufs=NSB * DC))
        p_x2t = ctx.enter_context(tc.tile_pool(name="x2ts", bufs=8))
        p_tt = ctx.enter_context(tc.tile_pool(name="tts", bufs=8))
        p_sc = ctx.enter_context(tc.tile_pool(name="scores", bufs=TC))
        p_pb = ctx.enter_context(tc.tile_pool(name="pbf", bufs=6))
        p_aux = ctx.enter_context(tc.tile_pool(name="aux", bufs=1))
        p_row = ctx.enter_context(tc.tile_pool(name="rows", bufs=1))
        p_out = ctx.enter_context(tc.tile_pool(name="outs", bufs=1))
        # PSUM: 4 banks for the output accumulators, 1 for the rowsum, 3
        # general-purpose (MM1/MM2/transposes/broadcasts) = 8 banks exactly.
        p_pso = ctx.enter_context(tc.tile_pool(name="pso", bufs=FPG, space="PSUM"))
        p_pssum = ctx.enter_context(tc.tile_pool(name="pssum", bufs=1, space="PSUM"))
        p_psg = ctx.enter_context(tc.tile_pool(name="psg", bufs=3, space="PSUM"))

        # ---------- constants (no DMA dependency) ----------
        identity = p_ones.tile([P, P], FP32, name="ident", tag="ident")
        nc.gpsimd.memset(identity[:], 0.0)
        nc.gpsimd.affine_select(
            out=identity[:], in_=identity[:], compare_op=ALU.not_equal,
            fill=1.0, base=0, pattern=[[-1, P]], channel_multiplier=1,
        )
        ones_row = p_ones.tile([1, P], FP32R, name="ones_row", tag="ones_row")
        nc.scalar.activation(ones_row[:], identity[0:1, :], AF.Identity, bias=1.0, scale=0.0)
        ones_col = p_ones.tile([P, 1], BF16, name="ones_col", tag="ones_col")
        nc.scalar.activation(ones_col[:], identity[:, 0:1], AF.Identity, bias=1.0, scale=0.0)

        # ---------- input DMAs, prepass-critical first ----------
        bias_cols = []
        for i in range(DC):
            b_t = p_bc.tile([P, 1], FP32R, name=f"bc{i}", tag="bc")
            nc.sync.dma_start(b_t[:], bias_d[i * P : (i + 1) * P, :])
            bias_cols.append(b_t)
        fcb_cols = []
        for i in range(FPG):
            c_t = p_fcb.tile([P, 1], FP32, name=f"fcb{i}", tag="fcb")
            nc.sync.dma_start(c_t[:], fcb_d[i * P : (i + 1) * P, :])
            fcb_cols.append(c_t)
        # fc_w chunks borrow the tT ring: they die in the prepass, before the
        # first tT tile is produced.
        fcw_tiles = []
        for i in range(DC):
            f_t = p_tt.tile([P, F], FP32R, name=f"fcw{i}", tag="tt")
            nc.sync.dma_start(f_t[:], fcwt_d[i * P : (i + 1) * P, :])
            fcw_tiles.append(f_t)
        # whole x1T, fp32r-resident for the entire kernel
        chunks = []
        for tg in range(NSB):
            row = []
            for ec in range(DC):
                c = p_x1tc.tile([P, SB], FP32R, name=f"x1tc_{tg}_{ec}", tag="x1tc")
                nc.sync.dma_start(
                    c[:], x1t_d[ec * P : (ec + 1) * P, tg * SB : (tg + 1) * SB]
                )
                row.append(c)
            chunks.append(row)
        u_tiles = []
        for i in range(DC):
            u_t = p_u.tile([P, D], FP32R, name=f"ur{i}", tag="ur")
            nc.sync.dma_start(u_t[:], u_d[i * P : (i + 1) * P, :])
            u_tiles.append(u_t)
        # superblock 0's x2T chunks, hoisted so the DMA overlaps the prepass
        x2t_sb0 = []
        for dc in range(DC):
            x2_t = p_x2t.tile([P, SB], FP32R, name=f"x2t_0_{dc}", tag="x2t")
            nc.sync.dma_start(x2_t[:], x2t_d[dc * P : (dc + 1) * P, 0:SB])
            x2t_sb0.append(x2_t)

        # ---------- prepass: kb = x1 @ bias, x1fc = x1 @ fc_w^T ----------
        kb_cols = [None] * TC
        x1fc_tiles = [None] * TC
        for tg in range(NSB):
            ps_kb = p_pssum.tile([1, SB], FP32, name=f"pskb{tg}", tag="pssum")
            for ec in range(DC):
                nc.tensor.matmul(
                    ps_kb[:], bias_cols[ec][:], chunks[tg][ec][:],
                    start=(ec == 0), stop=(ec == DC - 1),
                )
            kb_r = p_row.tile([1, SB], FP32R, name=f"kbrow{tg}", tag="kbrow")
            nc.vector.tensor_copy(kb_r[:], ps_kb[:])
            for sub in range(4):
                ti = tg * 4 + sub
                ps_x = p_psg.tile([P, F], FP32, name=f"psx{ti}", tag="psg")
                for ec in range(DC):
                    nc.tensor.matmul(
                        ps_x[:],
                        chunks[tg][ec][:, sub * P : (sub + 1) * P],
                        fcw_tiles[ec][:],
                        start=(ec == 0), stop=(ec == DC - 1),
                    )
                xf = p_x1fc.tile([P, F], BF16, name=f"x1fc{ti}", tag="x1fc")
                nc.vector.tensor_copy(xf[:], ps_x[:])
                x1fc_tiles[ti] = xf
                ps_c = p_psg.tile([P, 4], FP32, name=f"pskc{ti}", tag="psg")
                nc.tensor.matmul(
                    ps_c[:], kb_r[0:1, sub * P : (sub + 1) * P],
                    ones_row[0:1, 0:4], start=True, stop=True,
                )
                kb_c = p_kb.tile([P, 1], FP32, name=f"kb{ti}", tag="kb")
                nc.vector.tensor_copy(kb_c[:], ps_c[:, 0:1])
                kb_cols[ti] = kb_c

        # ---------- MM1 emitter: tT chunks for superblock sb ----------
        def emit_mm1(sb, part, x2t_tiles=None):
            s0 = sb * SB
            if x2t_tiles is None:
                if part == 0:
                    x2t_tiles = []
                    for dc in range(DC):
                        x2_t = p_x2t.tile(
                            [P, SB], FP32R, name=f"x2t_{sb}_{dc}", tag="x2t"
                        )
                        nc.sync.dma_start(
                            x2_t[:], x2t_d[dc * P : (dc + 1) * P, s0 : s0 + SB]
                        )
                        x2t_tiles.append(x2_t)
                    emit_mm1.x2t = x2t_tiles
                x2t_tiles = emit_mm1.x2t
            tt_tiles = []
            for g in (0, 1) if part == 0 else (2, 3):
                ps_t = [
                    p_psg.tile([P, SB], FP32, name=f"pst{sb}_{g}_{i}", tag="psg")
                    for i in range(2)
                ]
                for dc in range(DC):
                    for i in range(2):
                        et = g * 2 + i
                        nc.tensor.matmul(
                            ps_t[i][:],
                            u_tiles[dc][:, et * P : (et + 1) * P],
                            x2t_tiles[dc][:],
                            start=(dc == 0), stop=(dc == DC - 1),
                        )
                for i in range(2):
                    t_t = p_tt.tile([P, SB], FP32R, name=f"tt{sb}_{g}_{i}", tag="tt")
                    nc.vector.tensor_copy(t_t[:], ps_t[i][:])
                    tt_tiles.append(t_t)
            return tt_tiles

        emit_mm1.x2t = x2t_sb0
        next_tt = emit_mm1(0, 0, x2t_sb0) + emit_mm1(0, 1, x2t_sb0)

        for sb in range(NSB):
            s0 = sb * SB
            tt_tiles = next_tt

            # ---- MM2: scoresT tiles + running elementwise max ----
            sc_tiles = []
            maxacc = p_aux.tile([P, SB], FP32, name=f"maxacc{sb}", tag="maxacc")
            for ti in range(TC):
                tg, sub = ti // 4, ti % 4
                ps_s = p_psg.tile([P, SB], FP32, name=f"pss{sb}_{ti}", tag="psg")
                for ec in range(DC):
                    nc.tensor.matmul(
                        ps_s[:],
                        chunks[tg][ec][:, sub * P : (sub + 1) * P],
                        tt_tiles[ec][:],
                        start=(ec == 0), stop=(ec == DC - 1),
                    )
                s_t = p_sc.tile([P, SB], FP32, name=f"sc{sb}_{ti}", tag="sc")
                nc.scalar.copy(s_t[:], ps_s[:])
                if ti == 0:
                    nc.gpsimd.tensor_copy(maxacc[:], s_t[:])
                else:
                    nc.vector.tensor_max(maxacc[:], maxacc[:], s_t[:])
                sc_tiles.append(s_t)

            # first half of next superblock's MM1 covers the max-reduce
            # handoff (VectorE chase tail -> PE transposes)
            if sb + 1 < NSB:
                next_tt = emit_mm1(sb + 1, 0)

            # ---- per-s max over partitions: transpose + free reduce ----
            mrow = p_row.tile([1, SB], FP32R, name=f"mrow{sb}", tag="mrow")
            for blk in range(SB // P):
                ps_tr = p_psg.tile([P, P], FP32, name=f"ptr{sb}_{blk}", tag="psg")
                nc.tensor.transpose(
                    ps_tr[:], maxacc[:, blk * P : (blk + 1) * P], identity[:]
                )
                mcol = p_row.tile([P, 1], FP32, name=f"mcol{sb}_{blk}", tag="mcol")
                nc.vector.reduce_max(mcol[:], ps_tr[:], axis=AX.X)
                ps_rr = p_psg.tile([1, P], FP32, name=f"prr{sb}_{blk}", tag="psg")
                nc.tensor.transpose(ps_rr[:], mcol[:], identity[:])
                nc.vector.tensor_copy(mrow[:, blk * P : (blk + 1) * P], ps_rr[:])
            ps_mb = p_psg.tile([P, SB], FP32, name=f"pmb{sb}", tag="psg")
            nc.tensor.matmul(ps_mb[:], ones_row[:], mrow[:], start=True, stop=True)
            maxb = p_aux.tile([P, SB], FP32, name=f"maxb{sb}", tag="maxb")
            nc.vector.tensor_copy(maxb[:], ps_mb[:])

            # ---- exp phase, chased by the PE: rowsum + output matmuls ----
            ps_sum = p_pssum.tile([1, SB], FP32, name=f"pssm{sb}", tag="pssum")
            ps_o = [
                p_pso.tile([P, SB], FP32, name=f"pso{sb}_{ft}", tag="pso")
                for ft in range(FPG)
            ]
            for ti in range(TC):
                nc.gpsimd.tensor_sub(sc_tiles[ti][:], sc_tiles[ti][:], maxb[:])
                p_t = p_pb.tile([P, SB], BF16, name=f"pb{sb}_{ti}", tag="pb")
                nc.scalar.activation(
                    p_t[:], sc_tiles[ti][:], AF.Exp, bias=kb_cols[ti][:], scale=1.0
                )
                nc.tensor.matmul(
                    ps_sum[:], ones_col[:], p_t[:],
                    start=(ti == 0), stop=(ti == TC - 1),
                )
                for ft in range(FPG):
                    nc.tensor.matmul(
                        ps_o[ft][:],
                        x1fc_tiles[ti][:, ft * P : (ft + 1) * P],
                        p_t[:],
                        start=(ti == 0), stop=(ti == TC - 1),
                    )

            # second half of next superblock's MM1 covers the recip handoff
            if sb + 1 < NSB:
                next_tt = next_tt + emit_mm1(sb + 1, 1)

            # ---- recip + broadcast ----
            srow = p_row.tile([1, SB], FP32, name=f"srow{sb}", tag="srow")
            nc.vector.tensor_copy(srow[:], ps_sum[:])
            rrow = p_row.tile([1, SB], FP32R, name=f"rrow{sb}", tag="rrow")
            with nc.allow_low_precision(reason="recip feeds fp32r matmul; fp22 ok"):
                nc.vector.reciprocal(rrow[:], srow[:])
            ps_rb = p_psg.tile([P, SB], FP32, name=f"prb{sb}", tag="psg")
            nc.tensor.matmul(ps_rb[:], ones_row[:], rrow[:], start=True, stop=True)
            recipb = p_aux.tile([P, SB], FP32, name=f"recipb{sb}", tag="recipb")
            nc.vector.tensor_copy(recipb[:], ps_rb[:])

            # ---- normalize + bias + relu + store ----
            for ft in range(FPG):
                tmp = p_out.tile([P, SB], FP32, name=f"tmp{sb}_{ft}", tag="tmp")
                nc.vector.tensor_mul(tmp[:], ps_o[ft][:], recipb[:])
                o_out = p_out.tile([P, SB], FP32, name=f"oo{sb}_{ft}", tag="oo")
                nc.scalar.activation(
                    o_out[:], tmp[:], AF.Relu, bias=fcb_cols[ft][:], scale=1.0
                )
                nc.sync.dma_start(outt_d[ft * P : (ft + 1) * P, s0 : s0 + SB], o_out[:])

    nc.compile()
    return nc


_NC_CACHE = None


def _get_nc():
    global _NC_CACHE
    if _NC_CACHE is None:
        _NC_CACHE = build_nc()
    return _NC_CACHE


def make_in_maps(x1, x2, U, bias, fc_w, fc_b):
    x1 = np.ascontiguousarray(np.asarray(x1, dtype=np.float32))
    x2 = np.ascontiguousarray(np.asarray(x2, dtype=np.float32))
    U = np.ascontiguousarray(np.asarray(U, dtype=np.float32))
    bias = np.asarray(bias, dtype=np.float32).reshape(D, 1)
    fc_w = np.asarray(fc_w, dtype=np.float32)
    fc_b = np.asarray(fc_b, dtype=np.float32).reshape(F, 1)
    fcwt = np.ascontiguousarray(fc_w.T)
    in_maps = []
    for b in range(B):
        in_maps.append(
            {
                "x1t": np.ascontiguousarray(x1[b].T),
                "x2t": np.ascontiguousarray(x2[b].T),
                "u": U,
                "fcwt": fcwt,
                "bias": bias,
                "fcb": fc_b,
            }
        )
    return in_maps


def kernel(x1, x2, U, bias, fc_w, fc_b):
    from concourse.bass_utils import run_bass_kernel_spmd

    nc = _get_nc()
    in_maps = make_in_maps(x1, x2, U, bias, fc_w, fc_b)
    res = run_bass_kernel_spmd(nc, in_maps, core_ids=list(range(B)))
    out = np.stack([np.ascontiguousarray(r["outt"].T) for r in res.results])
    return out.astype(np.float32)
